# revision 15
# baseline (speedup 1.0000x reference)
"""TRN2 Bass kernel for nn_ClusteringLayer (vq_codebook).

Computes, for inputs x (131072, 256) and clusters c (256, 256):
    dist2[r,k] = ||x_r||^2 + ||c_k||^2 - 2 x_r.c_k
    q = 1/(1+dist2);  q = q / sum_k q          (ALPHA=1 -> power is a no-op)

Strategy (data-parallel over 8 NeuronCores, 16384 rows each):
  - x is fp8e4 (halves input DMA), clusters stay fp16: the PE allows mixed
    operand dtypes, and x2/c2 are host-computed from the SAME quantized
    operands so the distance is consistent (measured ~1.1e-2 max rel vs the
    2e-2 gate).
  - PE does ONLY the two 128-contraction cross-term matmuls per 128-row
    block (fp8 xT chunk stationary x fp16 -2c chunk moving). The PE weight
    path double-buffers, so the steady-state period is
    (matmul_dur + ldweights)/2 ~ 224ns; dropping the old rank-3 "aug"
    matmul (x2/c2/1 terms) cuts PE time by a third.
  - The affine terms ride the elementwise pass instead: a custom DVE op
    RECIP_SUM_AUG_ANT (registered below) computes, in ONE 8-stage pass,
        t = (psum + (1+x2_r)) + c2_k ; out = ~1/t ; accum_out = row-sum
    with (1+x2_r) as the per-partition s0 scalar, c2_k as the elementwise
    in1 operand (a host-prepared 128-row broadcast of the c2 row), and a
    bit-flip seed + one Newton step for the reciprocal (~0.4% one-sided,
    which largely cancels between numerator and row-sum; end-to-end
    simulated err stays ~1.1e-2).
  - Normalize q = qun * (1/s): blocks 0-8 on GPSIMD (normalize_recip),
    blocks 9-15 on ACT (activation Copy with per-partition 1/s scale, the
    1/s vector from one tiny DVE recip).
  - Output fp16, DRAM [p, st, b, k] so each half-supertile store is a
    4KB-per-partition contiguous run; host unscrambles to [r, k] fp32.
  - Matmult instructions carry only ONE sync-wait: PE-side consts live in
    one DMA'd tile fenced by one dummy matmul; each supertile's xt DMA is
    fenced the same way. A prologue dummy-matmul burst un-throttles the
    HAM clock gate; the main matmul stream is dense enough to stay warm.
"""

import os
import sys
from operator import add as _op_add

for _p in ("/root/.axon_site/_ro/trn_rl_repo", "/opt/trn_rl_repo"):
    if os.path.isdir(_p) and _p not in sys.path:
        sys.path.append(_p)

import numpy as np
import ml_dtypes

from concourse import bacc, tile
import concourse.mybir as mybir
from concourse.bass_utils import run_bass_kernel_spmd
import concourse.dve_ops as dve_ops
from concourse.dve_ops import DveOp, RECIP_APPROX_FAST_CONSTS
from concourse.dve_spec import Spec, Src0, Src1, C0, C1, C2, AluOp, Bin, Zero, lower
from concourse.dve_uop import DveOpSpec

F32 = mybir.dt.float32
F16 = mybir.dt.float16
F8 = mybir.dt.float8e4

NCORES = 8
B = 131072
D = 256
K = 256
R = B // NCORES          # rows per core
S = 2048                 # rows per supertile
NB = S // 128            # 128-row blocks per supertile (16)
NST = R // S             # supertiles per core (8)
GSZ = 512                # warmup matmul free-dim size
BPP = 2                  # blocks per PSUM tile ([128, BPP*256] = 1 bank)
NACT = 7                 # normalize blocks 0..NACT-1 on ACT (rest on GPSIMD)

KW = 512                 # konst tile: ct[p, ch*256+k] = -2*fp16(c)[k, ch*128+p]
WARMUP_MMS = 32


def _register_recip_sum_aug():
    """Custom DVE op: t = (in0 + s0_p) + in1; out = ~1/t (bit-flip seed +
    one Newton step); accum_out = row-sum(out). Body is 7 ALU stages so the
    accum fits in stage 8."""
    name = "RECIP_SUM_AUG_ANT"
    for op in dve_ops.OPS:
        if op.name == name:
            return op
    t = (Src0 + C0) + Src1
    y0 = Bin(AluOp.BITWISE_NOT, t, t) * C2
    body = y0 * (C1 - t * y0)

    def _ref(in0, in1, c0, c1, c2):
        t = (in0.astype(np.float32) + np.float32(c0)) + in1.astype(np.float32)
        not_t = (~t.view(np.int32)).view(np.float32)
        y0 = not_t * np.float32(c2)
        b = (y0 * (np.float32(c1) - t * y0)).astype(np.float32)
        return b, b.reshape(b.shape[0], -1).sum(axis=-1, keepdims=True)

    spec = Spec(body=body, accum=_op_add, accum_init=Zero, reference=_ref)
    op = DveOp(name, spec, subdim=False, uops_sha={})
    dve_ops.OPS.append(op)
    dve_ops.CUSTOM_DVE_SPECS[name] = spec
    dve_ops._SUB_OPCODE_FOR_NAME[name] = (
        dve_ops._CUSTOM_DVE_ROW_BASE + len(dve_ops.OPS) - 1)
    for ver in ("v3", "v4"):
        try:
            uops = lower(spec, ver=ver)
        except Exception:
            continue
        s = DveOpSpec(name=name, opcode=dve_ops.get_dve_sub_opcode(name),
                      uops=uops, rd1_en=True)
        op.uops_sha[ver] = s.sha(ver)
    return op


RECIP_SUM_AUG = _register_recip_sum_aug()
_RC = RECIP_APPROX_FAST_CONSTS

_nc_cache = None


def _build():
    nc = bacc.Bacc("TRN2", target_bir_lowering=False, debug=False,
                   num_devices=NCORES)
    xt_d = nc.dram_tensor("xt", [128, NST, 2, S], F8, kind="ExternalInput").ap()
    x2_d = nc.dram_tensor("x2", [128, NST * NB], F32, kind="ExternalInput").ap()
    c2_d = nc.dram_tensor("c2", [128, K], F32, kind="ExternalInput").ap()
    ko_d = nc.dram_tensor("ko", [128, KW], F16, kind="ExternalInput").ap()
    out_d = nc.dram_tensor("out", [128, NST, NB, K], F16,
                           kind="ExternalOutput").ap()

    with tile.TileContext(nc) as tc:
        with (
            tc.tile_pool(name="const", bufs=1) as cpool,
            tc.tile_pool(name="xtp", bufs=4) as xtpool,
            tc.tile_pool(name="qunp", bufs=3) as qunpool,
            tc.tile_pool(name="o16p", bufs=3) as o16pool,
            tc.tile_pool(name="sp", bufs=4) as spool,
            tc.tile_pool(name="rsp", bufs=4) as rspool,
            tc.tile_pool(name="qps", bufs=6, space="PSUM") as qpool,
            tc.tile_pool(name="x2ps", bufs=1, space="PSUM") as x2pool,
        ):
            ko_t = cpool.tile([128, KW], F16, tag="ko")
            nc.sync.dma_start(ko_t[:], ko_d[:])
            # x2+1 per row ([p, st*NB+b]) and the 128-row c2 broadcast, both
            # fp32, consumed only by the DVE custom op (sem waits are fine
            # on non-PE engines, no fence needed). Their dma_starts are
            # deferred until after supertile 0's xt so the first matmuls
            # aren't stuck behind them in the serial HWDGE issue queue.
            x2_t = cpool.tile([128, NST * NB], F32, tag="x2")
            c2_t = cpool.tile([128, K], F32, tag="c2")

            ct = ko_t[:, 0:512].rearrange("p (c k) -> p c k", c=2)

            # Prologue: one fence matmul absorbs the konst DMA wait. No
            # dummy warm-up burst: the real matmul stream is dense enough to
            # un-throttle the HAM clock gate by itself, and the PE has slack
            # against the DVE pace even at the cold 1.2 GHz clock.
            fence_p = x2pool.tile([1, GSZ], F32, tag="fence")
            nc.tensor.matmul(fence_p[0:1, 0:8], ko_t[:, 0:1], ko_t[:, 0:8],
                             start=True, stop=True)

            for st in range(NST):
                xt_t = xtpool.tile([128, 2, S], F8, tag="xt")
                nc.sync.dma_start(xt_t[:], xt_d[:, st])
                if st == 0:
                    nc.sync.dma_start(x2_t[:], x2_d[:])
                    nc.sync.dma_start(c2_t[:], c2_d[:])

                # per-supertile fence absorbs the xt DMA wait
                nc.tensor.matmul(fence_p[0:1, 0:8], xt_t[:, 0, 0:1],
                                 xt_t[:, 0, 0:8], start=True, stop=True)

                qun_t = qunpool.tile([128, NB, K], F32, tag="qun")
                o16_t = o16pool.tile([128, NB, K], F16, tag="o16")
                s_t = spool.tile([128, NB], F32, tag="s")
                rs_t = rspool.tile([128, NACT], F32, tag="rs")
                for bp in range(NB // BPP):
                    # BPP 128-row blocks share one 2-bank [128, BPP*256] psum
                    qp = qpool.tile([128, BPP, K], F32, tag="qp")
                    for h in range(BPP):
                        b = BPP * bp + h
                        for ch in range(2):
                            nc.tensor.matmul(
                                qp[:, h, :],
                                xt_t[:, ch, b * 128:(b + 1) * 128],
                                ct[:, ch, :],
                                start=(ch == 0), stop=(ch == 1),
                            )
                    for h in range(BPP):
                        b = BPP * bp + h
                        nc.vector._custom_dve(
                            RECIP_SUM_AUG,
                            out=qun_t[:, b, :], in0=qp[:, h, :], in1=c2_t[:],
                            s0=x2_t[:, st * NB + b:st * NB + b + 1],
                            s1=_RC["s1"], imm2=_RC["s0"],
                            accum_out=s_t[:, b:b + 1],
                        )
                        if b == NACT - 1:
                            # 1/s for the ACT-normalized blocks 0..NACT-1:
                            # issued mid-stream so ACT's copies overlap the
                            # remaining fused ops (GP computes its own 1/s)
                            nc.vector.reciprocal_approx_fast(
                                out=rs_t[:], in_=s_t[:, 0:NACT])
                        elif b >= NACT:
                            nc.gpsimd.normalize_recip(
                                o16_t[:, b, :], qun_t[:, b, :],
                                s_t[:, b:b + 1])

                for b in range(NACT):
                    nc.scalar.activation(
                        o16_t[:, b, :], qun_t[:, b, :],
                        mybir.ActivationFunctionType.Copy,
                        scale=rs_t[:, b:b + 1],
                    )

                # two half-supertile output DMAs: the first only waits on
                # normalize of blocks 0-7, shortening the drain tail
                for hh in range(2):
                    nc.sync.dma_start(
                        out_d[:, st, hh * (NB // 2):(hh + 1) * (NB // 2), :],
                        o16_t[:, hh * (NB // 2):(hh + 1) * (NB // 2), :],
                    )
    nc.compile()
    return nc


def _get_nc():
    global _nc_cache
    if _nc_cache is None:
        _nc_cache = _build()
    return _nc_cache


def _prep_in_maps(inputs, clusters):
    x = np.asarray(inputs, dtype=np.float32)
    c = np.asarray(clusters, dtype=np.float32)

    x8 = x.astype(ml_dtypes.float8_e4m3)
    # [core][p, st, ch, s] = x8[core*R + st*S + s, ch*128 + p]
    xt_all = np.ascontiguousarray(
        x8.reshape(NCORES, NST, S, 2, 128).transpose(0, 4, 1, 3, 2))
    # 1 + x2 per row (consistent with the fp8-rounded x), laid out
    # [core][p, st*NB+b] to match the block structure (row = st*S + b*128 + p)
    x2p1 = (1.0 + (x8.astype(np.float64) ** 2).sum(1)).astype(np.float32)
    x2_all = np.ascontiguousarray(
        x2p1.reshape(NCORES, NST, NB, 128).transpose(0, 3, 1, 2)
        .reshape(NCORES, 128, NST * NB))

    ch = c.astype(np.float16)
    c2h = (ch.astype(np.float64) ** 2).sum(1).astype(np.float32)
    c2br = np.ascontiguousarray(np.broadcast_to(c2h[None, :], (128, K)))

    ko = np.zeros((128, KW), np.float16)
    # ct: -2 * ch.T  (exact doubling in fp16)
    ko[:, 0:512] = np.ascontiguousarray(
        (-2.0 * ch.astype(np.float32)).astype(np.float16).T
    ).reshape(2, 128, K).transpose(1, 0, 2).reshape(128, 512)

    return [
        {"xt": xt_all[i], "x2": x2_all[i], "c2": c2br, "ko": ko}
        for i in range(NCORES)
    ]


def _run(inputs, clusters, trace=False, tmpdir=None):
    nc = _get_nc()
    in_maps = _prep_in_maps(inputs, clusters)
    res = run_bass_kernel_spmd(nc, in_maps, list(range(NCORES)),
                               trace=trace, tmpdir=tmpdir)
    # device out: [128, NST, NB, K] fp16 with q[st*S + b*128 + p, k]
    out = np.concatenate(
        [np.asarray(res.results[i]["out"])
         .transpose(1, 2, 0, 3).reshape(R, K) for i in range(NCORES)],
        axis=0).astype(np.float32)
    return out, res


def kernel(inputs, clusters):
    out, _ = _run(inputs, clusters, trace=False)
    return out


# revision 18
# speedup vs baseline: 1.0186x; 1.0186x over previous
"""TRN2 Bass kernel for nn_ClusteringLayer (vq_codebook).

Computes, for inputs x (131072, 256) and clusters c (256, 256):
    dist2[r,k] = ||x_r||^2 + ||c_k||^2 - 2 x_r.c_k
    q = 1/(1+dist2);  q = q / sum_k q          (ALPHA=1 -> power is a no-op)

Strategy (data-parallel over 8 NeuronCores, 16384 rows each):
  - x is fp8e4 (halves input DMA), clusters stay fp16: the PE allows mixed
    operand dtypes, and x2/c2 are host-computed from the SAME quantized
    operands so the distance is consistent (measured ~1.1e-2 max rel vs the
    2e-2 gate).
  - PE does ONLY the two 128-contraction cross-term matmuls per 128-row
    block (fp8 xT chunk stationary x fp16 -2c chunk moving). The PE weight
    path double-buffers, so the steady-state period is
    (matmul_dur + ldweights)/2 ~ 224ns; dropping the old rank-3 "aug"
    matmul (x2/c2/1 terms) cuts PE time by a third.
  - The affine terms ride the elementwise pass instead: a custom DVE op
    RECIP_SUM_AUG_ANT (registered below) computes, in ONE 8-stage pass,
        t = (psum + (1+x2_r)) + c2_k ; out = ~1/t ; accum_out = row-sum
    with (1+x2_r) as the per-partition s0 scalar, c2_k as the elementwise
    in1 operand (a host-prepared 128-row broadcast of the c2 row), and a
    bit-flip seed + one Newton step for the reciprocal (~0.4% one-sided,
    which largely cancels between numerator and row-sum; end-to-end
    simulated err stays ~1.1e-2).
  - Normalize q = qun * (1/s): blocks 0-6 on ACT (activation Copy with
    per-partition 1/s scale; its 1/s batch comes from one tiny DVE recip
    issued mid-stream so the copies overlap the remaining fused ops),
    blocks 7-15 on GPSIMD (normalize_recip, which derives its own 1/s).
  - Output fp16, DRAM [p, st, b, k] so each half-supertile store is a
    4KB-per-partition contiguous run; host unscrambles to [r, k] fp32.
  - Matmult instructions carry only ONE sync-wait: PE-side consts live in
    one DMA'd tile fenced by one dummy matmul; each supertile's xt DMA is
    fenced the same way. No dummy warm-up burst: PE has slack against the
    DVE pace even at the cold HAM clock, so warming with real work is a
    net win.

Measured on HW: ~75us vs the 131us session baseline; max rel err 1.07e-2
(gate 2e-2), dominated by the fp8 quantization of x (9.8e-3).
"""

import os
import sys
from operator import add as _op_add

for _p in ("/root/.axon_site/_ro/trn_rl_repo", "/opt/trn_rl_repo"):
    if os.path.isdir(_p) and _p not in sys.path:
        sys.path.append(_p)

import numpy as np
import ml_dtypes

from concourse import bacc, tile
import concourse.mybir as mybir
from concourse.bass_utils import run_bass_kernel_spmd
import concourse.dve_ops as dve_ops
from concourse.dve_ops import DveOp, RECIP_APPROX_FAST_CONSTS
from concourse.dve_spec import Spec, Src0, Src1, C0, C1, C2, AluOp, Bin, Zero, lower
from concourse.dve_uop import DveOpSpec

F32 = mybir.dt.float32
F16 = mybir.dt.float16
F8 = mybir.dt.float8e4

NCORES = 8
B = 131072
D = 256
K = 256
R = B // NCORES          # rows per core
S = 2048                 # rows per supertile
NB = S // 128            # 128-row blocks per supertile (16)
NST = R // S             # supertiles per core (8)
GSZ = 512                # warmup matmul free-dim size
BPP = 2                  # blocks per PSUM tile ([128, BPP*256] = 1 bank)
NACT = 7                 # normalize blocks 0..NACT-1 on ACT (rest on GPSIMD)

KW = 512                 # konst tile: ct[p, ch*256+k] = -2*fp16(c)[k, ch*128+p]


def _register_recip_sum_aug():
    """Custom DVE op: t = (in0 + s0_p) + in1; out = ~1/t (bit-flip seed +
    one Newton step); accum_out = row-sum(out). Body is 7 ALU stages so the
    accum fits in stage 8."""
    name = "RECIP_SUM_AUG_ANT"
    for op in dve_ops.OPS:
        if op.name == name:
            return op
    t = (Src0 + C0) + Src1
    y0 = Bin(AluOp.BITWISE_NOT, t, t) * C2
    body = y0 * (C1 - t * y0)

    def _ref(in0, in1, c0, c1, c2):
        t = (in0.astype(np.float32) + np.float32(c0)) + in1.astype(np.float32)
        not_t = (~t.view(np.int32)).view(np.float32)
        y0 = not_t * np.float32(c2)
        b = (y0 * (np.float32(c1) - t * y0)).astype(np.float32)
        return b, b.reshape(b.shape[0], -1).sum(axis=-1, keepdims=True)

    spec = Spec(body=body, accum=_op_add, accum_init=Zero, reference=_ref)
    op = DveOp(name, spec, subdim=False, uops_sha={})
    dve_ops.OPS.append(op)
    dve_ops.CUSTOM_DVE_SPECS[name] = spec
    dve_ops._SUB_OPCODE_FOR_NAME[name] = (
        dve_ops._CUSTOM_DVE_ROW_BASE + len(dve_ops.OPS) - 1)
    for ver in ("v3", "v4"):
        try:
            uops = lower(spec, ver=ver)
        except Exception:
            continue
        s = DveOpSpec(name=name, opcode=dve_ops.get_dve_sub_opcode(name),
                      uops=uops, rd1_en=True)
        op.uops_sha[ver] = s.sha(ver)
    return op


RECIP_SUM_AUG = _register_recip_sum_aug()
_RC = RECIP_APPROX_FAST_CONSTS

_nc_cache = None


def _build():
    nc = bacc.Bacc("TRN2", target_bir_lowering=False, debug=False,
                   num_devices=NCORES)
    xt_d = nc.dram_tensor("xt", [128, NST, 2, S], F8, kind="ExternalInput").ap()
    x2_d = nc.dram_tensor("x2", [128, NST * NB], F32, kind="ExternalInput").ap()
    c2_d = nc.dram_tensor("c2", [128, K], F32, kind="ExternalInput").ap()
    ko_d = nc.dram_tensor("ko", [128, KW], F16, kind="ExternalInput").ap()
    out_d = nc.dram_tensor("out", [128, NST, NB, K], F16,
                           kind="ExternalOutput").ap()

    with tile.TileContext(nc) as tc:
        with (
            tc.tile_pool(name="const", bufs=1) as cpool,
            tc.tile_pool(name="xtp", bufs=4) as xtpool,
            tc.tile_pool(name="qunp", bufs=3) as qunpool,
            tc.tile_pool(name="o16p", bufs=3) as o16pool,
            tc.tile_pool(name="sp", bufs=4) as spool,
            tc.tile_pool(name="rsp", bufs=4) as rspool,
            tc.tile_pool(name="qps", bufs=7, space="PSUM") as qpool,
            tc.tile_pool(name="x2ps", bufs=1, space="PSUM") as x2pool,
        ):
            ko_t = cpool.tile([128, KW], F16, tag="ko")
            nc.sync.dma_start(ko_t[:], ko_d[:])
            # x2+1 per row ([p, st*NB+b]) and the 128-row c2 broadcast, both
            # fp32, consumed only by the DVE custom op (sem waits are fine
            # on non-PE engines, no fence needed). Their dma_starts are
            # deferred until after supertile 0's xt so the first matmuls
            # aren't stuck behind them in the serial HWDGE issue queue.
            x2_t = cpool.tile([128, NST * NB], F32, tag="x2")
            c2_t = cpool.tile([128, K], F32, tag="c2")

            ct = ko_t[:, 0:512].rearrange("p (c k) -> p c k", c=2)

            # Prologue: one fence matmul absorbs the konst DMA wait. No
            # dummy warm-up burst: the real matmul stream is dense enough to
            # un-throttle the HAM clock gate by itself, and the PE has slack
            # against the DVE pace even at the cold 1.2 GHz clock.
            fence_p = x2pool.tile([1, GSZ], F32, tag="fence")
            nc.tensor.matmul(fence_p[0:1, 0:8], ko_t[:, 0:1], ko_t[:, 0:8],
                             start=True, stop=True)

            for st in range(NST):
                xt_t = xtpool.tile([128, 2, S], F8, tag="xt")
                nc.sync.dma_start(xt_t[:], xt_d[:, st])
                if st == 0:
                    nc.sync.dma_start(x2_t[:], x2_d[:])
                    nc.sync.dma_start(c2_t[:], c2_d[:])

                # per-supertile fence absorbs the xt DMA wait
                nc.tensor.matmul(fence_p[0:1, 0:8], xt_t[:, 0, 0:1],
                                 xt_t[:, 0, 0:8], start=True, stop=True)

                qun_t = qunpool.tile([128, NB, K], F32, tag="qun")
                o16_t = o16pool.tile([128, NB, K], F16, tag="o16")
                s_t = spool.tile([128, NB], F32, tag="s")
                rs_t = rspool.tile([128, NACT], F32, tag="rs")
                for bp in range(NB // BPP):
                    # BPP 128-row blocks share one 2-bank [128, BPP*256] psum
                    qp = qpool.tile([128, BPP, K], F32, tag="qp")
                    for h in range(BPP):
                        b = BPP * bp + h
                        for ch in range(2):
                            nc.tensor.matmul(
                                qp[:, h, :],
                                xt_t[:, ch, b * 128:(b + 1) * 128],
                                ct[:, ch, :],
                                start=(ch == 0), stop=(ch == 1),
                            )
                    for h in range(BPP):
                        b = BPP * bp + h
                        nc.vector._custom_dve(
                            RECIP_SUM_AUG,
                            out=qun_t[:, b, :], in0=qp[:, h, :], in1=c2_t[:],
                            s0=x2_t[:, st * NB + b:st * NB + b + 1],
                            s1=_RC["s1"], imm2=_RC["s0"],
                            accum_out=s_t[:, b:b + 1],
                        )
                        if b == NACT - 1:
                            # 1/s for the ACT-normalized blocks 0..NACT-1:
                            # issued mid-stream so ACT's copies overlap the
                            # remaining fused ops (GP computes its own 1/s)
                            nc.vector.reciprocal_approx_fast(
                                out=rs_t[:], in_=s_t[:, 0:NACT])
                        elif b >= NACT:
                            nc.gpsimd.normalize_recip(
                                o16_t[:, b, :], qun_t[:, b, :],
                                s_t[:, b:b + 1])

                for b in range(NACT):
                    nc.scalar.activation(
                        o16_t[:, b, :], qun_t[:, b, :],
                        mybir.ActivationFunctionType.Copy,
                        scale=rs_t[:, b:b + 1],
                    )

                # two half-supertile output DMAs: the first only waits on
                # normalize of blocks 0-7, shortening the drain tail
                for hh in range(2):
                    nc.sync.dma_start(
                        out_d[:, st, hh * (NB // 2):(hh + 1) * (NB // 2), :],
                        o16_t[:, hh * (NB // 2):(hh + 1) * (NB // 2), :],
                    )
    nc.compile()
    return nc


def _get_nc():
    global _nc_cache
    if _nc_cache is None:
        _nc_cache = _build()
    return _nc_cache


def _prep_in_maps(inputs, clusters):
    x = np.asarray(inputs, dtype=np.float32)
    c = np.asarray(clusters, dtype=np.float32)

    x8 = x.astype(ml_dtypes.float8_e4m3)
    # [core][p, st, ch, s] = x8[core*R + st*S + s, ch*128 + p]
    xt_all = np.ascontiguousarray(
        x8.reshape(NCORES, NST, S, 2, 128).transpose(0, 4, 1, 3, 2))
    # 1 + x2 per row (consistent with the fp8-rounded x), laid out
    # [core][p, st*NB+b] to match the block structure (row = st*S + b*128 + p)
    x2p1 = (1.0 + (x8.astype(np.float64) ** 2).sum(1)).astype(np.float32)
    x2_all = np.ascontiguousarray(
        x2p1.reshape(NCORES, NST, NB, 128).transpose(0, 3, 1, 2)
        .reshape(NCORES, 128, NST * NB))

    ch = c.astype(np.float16)
    c2h = (ch.astype(np.float64) ** 2).sum(1).astype(np.float32)
    c2br = np.ascontiguousarray(np.broadcast_to(c2h[None, :], (128, K)))

    ko = np.zeros((128, KW), np.float16)
    # ct: -2 * ch.T  (exact doubling in fp16)
    ko[:, 0:512] = np.ascontiguousarray(
        (-2.0 * ch.astype(np.float32)).astype(np.float16).T
    ).reshape(2, 128, K).transpose(1, 0, 2).reshape(128, 512)

    return [
        {"xt": xt_all[i], "x2": x2_all[i], "c2": c2br, "ko": ko}
        for i in range(NCORES)
    ]


def _run(inputs, clusters, trace=False, tmpdir=None):
    nc = _get_nc()
    in_maps = _prep_in_maps(inputs, clusters)
    res = run_bass_kernel_spmd(nc, in_maps, list(range(NCORES)),
                               trace=trace, tmpdir=tmpdir)
    # device out: [128, NST, NB, K] fp16 with q[st*S + b*128 + p, k]
    out = np.concatenate(
        [np.asarray(res.results[i]["out"])
         .transpose(1, 2, 0, 3).reshape(R, K) for i in range(NCORES)],
        axis=0).astype(np.float32)
    return out, res


def kernel(inputs, clusters):
    out, _ = _run(inputs, clusters, trace=False)
    return out
